# revision 17
# baseline (speedup 1.0000x reference)
"""GroupQueryAttention TRN2 Bass kernel.

Problem: B=4, T=2048, C=1024, H=16 heads, G=4 groups, head_dim=64, causal.
Sharding: 8 cores = 4 batches (DP) x 2 tensor-parallel halves (8 heads /
2 groups each). Host pre-transposes x and weight slices; each core computes
a partial output projection over its 512 attention channels; host sums the
two TP partials per batch and adds the bias.

Device algorithm (per core, all matmuls fp32r):
  qT[h] = WqT_h.T @ xT   (pair-packed: 2 heads per 128-partition tile)
  kT[g], vT[g] likewise; v transposed to [T, 64] via PE, augmented with a
  ones column so the PV matmul also produces the softmax denominators.
  scoresT[tk, tq] = kT.T-block @ qT-block  (causal: skip/clip blocks)
  pT = exp(scoresT * 0.125)  (ACT, PSUM->SBUF; no max-subtraction needed --
  scores are O(1));  diagonal 128x128 blocks masked by an upper-tri 0/1 mask.
  outT[65, tq] += v_aug.T @ pT  ->  row 64 = sum_k p  (denominator)
  normalize: recip = 1/denom (DVE); broadcast via DRAM round-trip DMA;
  attnT = outT * recip_bcast;  y[tq, :] += attnT.T @ WpT (partial, fp32 out).
"""

import sys
import numpy as np
import ml_dtypes

for _p in ("/opt/trn_rl_repo", "/opt/trn_rl_repo/concourse"):
    if _p not in sys.path:
        sys.path.insert(0, _p)

import concourse.bass as bass  # noqa: E402
import concourse.mybir as mybir  # noqa: E402
from concourse import bacc  # noqa: E402
from concourse.tile import TileContext  # noqa: E402
from concourse.bass_utils import run_bass_kernel_spmd  # noqa: E402
from concourse.masks import make_identity, make_upper_triangular  # noqa: E402

F32 = mybir.dt.float32
F32R = mybir.dt.float32r
BF16 = mybir.dt.bfloat16

B, T, C = 4, 2048, 1024
NH, NG, HD = 16, 4, 64
NH_LOC, NG_LOC = 8, 2          # per-core heads / groups
S = NH_LOC * HD                # 512 local attention channels
TQB = 512                      # tq block
NTQB = T // TQB                # 4
NKT = T // 128                 # 16 tk tiles
NCT = C // 128                 # 8 contraction tiles
SCALE = float(HD) ** -0.5


def _build_program(trace_scopes=False):
    nc = bacc.Bacc("TRN2", target_bir_lowering=False, debug=False, num_devices=8)

    xT = nc.dram_tensor("xT", [C, T], F32R, kind="ExternalInput")
    wqT = nc.dram_tensor("wqT", [C, S], F32R, kind="ExternalInput")
    wkT = nc.dram_tensor("wkT", [C, NG_LOC * HD], F32R, kind="ExternalInput")
    wvT = nc.dram_tensor("wvT", [C, NG_LOC * HD], F32R, kind="ExternalInput")
    wpT = nc.dram_tensor("wpT", [S, C], F32R, kind="ExternalInput")
    y = nc.dram_tensor("y", [T, C], F32, kind="ExternalOutput")

    with TileContext(nc) as tc:
        with tc.tile_pool(name="const", bufs=1) as const_pool, \
             tc.tile_pool(name="persist", bufs=1) as persist, \
             tc.tile_pool(name="dram", bufs=4, space="DRAM") as dram_pool:

            ident = const_pool.tile([128, 64], F32)
            make_identity(nc, ident[0:64, 0:64])
            make_identity(nc, ident[64:128, 0:64], nomemset=False)
            mask32 = const_pool.tile([128, 128], F32)
            make_upper_triangular(nc, mask32, val=1.0, diag=True)
            mask = const_pool.tile([128, 128], BF16)
            nc.vector.tensor_copy(mask, mask32)
            ones64 = const_pool.tile([128, 64], F32)
            nc.vector.memset(ones64, 1.0)

            # ---- persistent SBUF tensors ----
            qt_sb = [persist.tile([128, T], BF16, tag=f"qt{i}", name=f"qt{i}") for i in range(4)]
            kdup = [persist.tile([128, T], BF16, tag=f"kd{g}", name=f"kd{g}")
                    for g in range(NG_LOC)]
            # v (transposed back): per group 16 tiles [128, 128]; first 64
            # lhsT cols are ones so PV emits 64 replicated denominator rows
            v_sb = [persist.tile([128, NKT * 128], BF16, tag=f"v{g}", name=f"v{g}")
                    for g in range(NG_LOC)]
            wp_sb = [persist.tile([128, C], F32R, tag=f"wp{i}", name=f"wp{i}") for i in range(4)]
            for i in range(4):
                nc.sync.dma_start(out=wp_sb[i], in_=wpT[i * 128:(i + 1) * 128, :])
            for g in range(NG_LOC):
                for t in range(NKT):
                    nc.vector.tensor_copy(
                        v_sb[g][:, t * 128:t * 128 + 64], ones64)

            # ================= Phase A: projections =================
            with tc.tile_pool(name="xw", bufs=1) as xw, \
                 tc.tile_pool(name="psA", bufs=4, space="PSUM") as psA:
                xts = [xw.tile([128, T], F32R, tag=f"x{ct}", name=f"x{ct}") for ct in range(NCT)]
                wq_sb = [xw.tile([128, S], F32R, tag=f"wq{ct}", name=f"wq{ct}") for ct in range(NCT)]
                wk_sb = [xw.tile([128, NG_LOC * HD], F32R, tag=f"wk{ct}", name=f"wk{ct}")
                         for ct in range(NCT)]
                wv_sb = [xw.tile([128, NG_LOC * HD], F32R, tag=f"wv{ct}", name=f"wv{ct}")
                         for ct in range(NCT)]
                for ct in range(NCT):
                    rows = slice(ct * 128, (ct + 1) * 128)
                    nc.sync.dma_start(out=xts[ct], in_=xT[rows, :])
                    nc.sync.dma_start(out=wq_sb[ct], in_=wqT[rows, :])
                    nc.sync.dma_start(out=wk_sb[ct], in_=wkT[rows, :])
                    nc.sync.dma_start(out=wv_sb[ct], in_=wvT[rows, :])

                # qT: 4 head-pairs x 4 tq blocks, accumulate over 8 c-tiles
                for p4 in range(4):
                    for j in range(NTQB):
                        ps = psA.tile([128, TQB], F32, tag="psA")
                        for ct in range(NCT):
                            nc.tensor.matmul(
                                ps,
                                wq_sb[ct][:, p4 * 128:(p4 + 1) * 128],
                                xts[ct][:, j * TQB:(j + 1) * TQB],
                                start=(ct == 0), stop=(ct == NCT - 1))
                        nc.scalar.copy(qt_sb[p4][:, j * TQB:(j + 1) * TQB], ps)

                # kT: one pair (2 groups); duplicate each group onto both
                # partition halves (matmul operands must share base_partition)
                for j in range(NTQB):
                    ps = psA.tile([128, TQB], F32, tag="psA")
                    for ct in range(NCT):
                        nc.tensor.matmul(
                            ps, wk_sb[ct], xts[ct][:, j * TQB:(j + 1) * TQB],
                            start=(ct == 0), stop=(ct == NCT - 1))
                    cols = slice(j * TQB, (j + 1) * TQB)
                    nc.scalar.copy(kdup[0][0:64, cols], ps[0:64, :])
                    nc.scalar.copy(kdup[1][64:128, cols], ps[64:128, :])
                nc.sync.dma_start(out=kdup[0][64:128, :], in_=kdup[0][0:64, :])
                nc.sync.dma_start(out=kdup[1][0:64, :], in_=kdup[1][64:128, :])

                # vT then PE-transpose into v_sb ([T,64] layout + ones col)
                vt_sb = xw.tile([128, T], F32, tag="vt")
                for j in range(NTQB):
                    ps = psA.tile([128, TQB], F32, tag="psA")
                    for ct in range(NCT):
                        nc.tensor.matmul(
                            ps, wv_sb[ct], xts[ct][:, j * TQB:(j + 1) * TQB],
                            start=(ct == 0), stop=(ct == NCT - 1))
                    nc.vector.tensor_copy(vt_sb[:, j * TQB:(j + 1) * TQB], ps)
                for g in range(NG_LOC):
                    for t in range(NKT):
                        pst = psA.tile([128, TQB], F32, tag="psA")
                        nc.tensor.transpose(
                            pst[:, 0:64],
                            vt_sb[g * 64:(g + 1) * 64, t * 128:(t + 1) * 128],
                            ident[g * 64:(g + 1) * 64, 0:64])
                        nc.vector.tensor_copy(
                            v_sb[g][:, t * 128 + 64:t * 128 + 128], pst[:, 0:64])

            # ================= Phase B: attention + proj =================
            with tc.tile_pool(name="pp", bufs=8) as ppool, \
                 tc.tile_pool(name="attn", bufs=8) as apool, \
                 tc.tile_pool(name="sm", bufs=4) as small, \
                 tc.tile_pool(name="yo", bufs=4) as ypool, \
                 tc.tile_pool(name="psS", bufs=4, space="PSUM") as psS, \
                 tc.tile_pool(name="psO", bufs=2, space="PSUM") as psO, \
                 tc.tile_pool(name="psP", bufs=2, space="PSUM") as psP:

                for j in range(NTQB):
                    tq0 = j * TQB
                    ntk = 4 * (j + 1)
                    at_j = [apool.tile([128, TQB], F32R, tag=f"at{p4}", name=f"at{p4}")
                            for p4 in range(4)]
                    for h in range(NH_LOC):
                        g = h // 4
                        p4, r = h // 2, h % 2
                        qT_h = qt_sb[p4][r * 64:(r + 1) * 64, :]
                        kT_g = kdup[g][r * 64:(r + 1) * 64, :]
                        po = psO.tile([128, TQB], F32, tag="po")
                        for t in range(ntk):
                            c = t - 4 * j
                            off = max(0, c * 128)
                            pscore = psS.tile([128, TQB], F32, tag="ps")
                            nc.tensor.matmul(
                                pscore[:, off:TQB],
                                kT_g[:, t * 128:(t + 1) * 128],
                                qT_h[:, tq0 + off:tq0 + TQB],
                                start=True, stop=True)
                            pt = ppool.tile([128, TQB], BF16, tag="pt")
                            nc.scalar.activation(
                                pt[:, off:TQB], pscore[:, off:TQB],
                                mybir.ActivationFunctionType.Exp, scale=SCALE)
                            if c >= 0:
                                nc.vector.tensor_mul(
                                    pt[:, off:off + 128],
                                    pt[:, off:off + 128], mask)
                            nc.tensor.matmul(
                                po[:, off:TQB],
                                v_sb[g][:, t * 128:(t + 1) * 128],
                                pt[:, off:TQB],
                                start=(t == 0), stop=(t == ntk - 1))
                        # normalization: recip -> DRAM -> broadcast -> mul
                        rcp = small.tile([128, TQB], F32, tag="recip")
                        nc.vector.reciprocal_approx_fast(rcp[0:64, :], po[0:64, :])
                        nc.vector.tensor_mul(
                            at_j[p4][r * 64:(r + 1) * 64, :],
                            po[64:128, :], rcp[0:64, :])
                    # output projection for this tq block
                    for tt in range(4):
                        tau = j * 4 + tt
                        ysb = ypool.tile([128, C], F32, tag="y")
                        for half in range(2):
                            yp = psP.tile([128, TQB], F32, tag="yp")
                            for p4 in range(4):
                                nc.tensor.matmul(
                                    yp,
                                    at_j[p4][:, tt * 128:(tt + 1) * 128],
                                    wp_sb[p4][:, half * TQB:(half + 1) * TQB],
                                    start=(p4 == 0), stop=(p4 == 3))
                            nc.vector.tensor_copy(
                                ysb[:, half * TQB:(half + 1) * TQB], yp)
                        nc.sync.dma_start(
                            out=y[tau * 128:(tau + 1) * 128, :], in_=ysb)

    nc.compile()
    return nc


_NC_CACHE = None


def _get_nc():
    global _NC_CACHE
    if _NC_CACHE is None:
        _NC_CACHE = _build_program()
    return _NC_CACHE


def _make_in_maps(x, Wq, Wk, Wv, Wp):
    in_maps = []
    for core in range(8):
        b, tp = core // 2, core % 2
        hs = slice(tp * NH_LOC, (tp + 1) * NH_LOC)
        gs = slice(tp * NG_LOC, (tp + 1) * NG_LOC)
        in_maps.append({
            "xT": np.ascontiguousarray(x[b].T),
            "wqT": np.ascontiguousarray(
                Wq[hs].transpose(2, 0, 1).reshape(C, S)),
            "wkT": np.ascontiguousarray(
                Wk[gs].transpose(2, 0, 1).reshape(C, NG_LOC * HD)),
            "wvT": np.ascontiguousarray(
                Wv[gs].transpose(2, 0, 1).reshape(C, NG_LOC * HD)),
            "wpT": np.ascontiguousarray(Wp[:, tp * S:(tp + 1) * S].T),
        })
    return in_maps


def kernel(x, Wq, Wk, Wv, Wp, bp, _trace=False):
    x = np.asarray(x, dtype=np.float32)
    nc = _get_nc()
    in_maps = _make_in_maps(
        x, np.asarray(Wq, np.float32), np.asarray(Wk, np.float32),
        np.asarray(Wv, np.float32), np.asarray(Wp, np.float32))
    res = run_bass_kernel_spmd(nc, in_maps, list(range(8)), trace=_trace)
    out = np.empty((B, T, C), dtype=np.float32)
    bp32 = np.asarray(bp, np.float32)
    for b in range(B):
        out[b] = res.results[2 * b]["y"] + res.results[2 * b + 1]["y"] + bp32
    if _trace:
        return out, res
    return out
